# revision 5
# baseline (speedup 1.0000x reference)
"""Locally-connected 1D conv (Conv1dLocal) on 8 Trainium2 NeuronCores.

out[b,o,s] = sum_{i,k} x[b,i,s+k] * w[o,i,s,k]
  x: (32, 64, 518) f32, weight: (64, 64, 512, 7) f32 -> out: (32, 64, 512) f32

Sharding: output positions s across 8 cores (64 each). Per core the conv is
phrased as 32 position-PAIR matmuls with full 128x128 PE utilization:
  lhsT chunk  [K=128 (2 timesteps x 64 in_ch), M=128 (2 positions x 64 out_ch)]
  rhs  block  [K=128, N=32 (batch)]
  psum [128, 32] accumulated over 4 chunks (taps zero-padded at the pair edges).
Consecutive x-blocks slide by one pair, so block t2 is shared by pairs
p = t2-3..t2 and no x data is duplicated.
"""

import numpy as np
import ml_dtypes

B, IC, OC, S, KW, L = 32, 64, 64, 512, 7, 518
NCORES = 8
SP = S // NCORES       # 64 positions per core
NP = SP // 2           # 32 pairs per core
NCHUNK = 4
NB = NP + NCHUNK - 1   # 35 x-blocks per core
WG = 4                 # pairs per weight-DMA slab

MODE = "f32"           # "f32" | "bf16x3" | "bf16"
REPEAT = 1

_cache = {}


# ---------------------------------------------------------------- host side

def _host_prep(x, weight, mode):
    """Build per-core DRAM images. Returns list of in_maps."""
    f32 = np.float32
    xt = np.ascontiguousarray(x.transpose(1, 2, 0))          # (IC, L, B)
    wpad = np.zeros((OC, IC, S, KW + 2), f32)
    wpad[..., 1:8] = weight

    in_maps = []
    for c in range(NCORES):
        s0 = SP * c
        t_idx = s0 + 2 * np.arange(NB)[None, :] + np.arange(2)[:, None]
        xb = xt[:, t_idx, :]                                  # (IC, 2, NB, B)
        xb = np.ascontiguousarray(xb.transpose(1, 0, 2, 3)).reshape(128, NB, B)

        wlay = np.empty((128, NP, NCHUNK, 128), f32)
        for ph in (0, 1):
            S_i = s0 + 2 * np.arange(NP) + ph
            K_i = (1 + 2 * np.arange(NCHUNK)[:, None]
                   + np.arange(2)[None, :] - ph)              # (4,2) [j,dt]
            sel = wpad[:, :, S_i, :]                          # (OC, IC, NP, 9)
            blk = sel[:, :, :, K_i]                           # (OC, IC, NP, 4, 2)
            wlay[:, :, :, ph * 64:(ph + 1) * 64] = (
                blk.transpose(4, 1, 2, 3, 0).reshape(128, NP, NCHUNK, OC))

        if mode == "f32":
            in_maps.append({"xb": xb, "wl": np.ascontiguousarray(wlay)})
        elif mode == "bf16":
            bf = ml_dtypes.bfloat16
            in_maps.append({"xh": xb.astype(bf), "wh": wlay.astype(bf)})
        elif mode == "bf16x3":
            bf = ml_dtypes.bfloat16
            xh = xb.astype(bf)
            xl = (xb - xh.astype(f32)).astype(bf)
            wh = wlay.astype(bf)
            wl = (wlay - wh.astype(f32)).astype(bf)
            in_maps.append({"xh": xh, "xl": xl, "wh": wh, "wl": wl})
        else:
            raise ValueError(mode)
    return in_maps


def _host_post(results):
    out = np.empty((B, OC, S), np.float32)
    for c in range(NCORES):
        od = results[c]["out"].reshape(2, OC, NP, B)
        out[:, :, SP * c: SP * (c + 1)] = (
            od.transpose(3, 1, 2, 0).reshape(B, OC, SP))
    return out


# ---------------------------------------------------------------- bass side

def _legalize_single_wait(nc):
    """This container's walrus accepts only ONE sync-wait per instruction.
    Hoist extra waits into standalone EventSemaphore insts on the same engine
    (sequential waits before the instruction are semantically identical)."""
    import concourse.mybir as mybir

    n = 0
    for f in nc.m.functions:
        for bb in f.blocks:
            out = []
            changed = False
            for inst in bb.instructions:
                si = inst.sync_info
                if si is not None and len(si.on_wait) > 1:
                    waits = list(si.on_wait)
                    for w in waits[:-1]:
                        ev = mybir.InstEventSemaphore(
                            name=f"LWAIT-{n}", ins=[], outs=[])
                        n += 1
                        ev.engine = inst.engine
                        ev.sync_info = mybir.SyncInfo(on_wait=[w], on_update=[])
                        out.append(ev)
                    inst.sync_info = mybir.SyncInfo(
                        on_wait=[waits[-1]], on_update=list(si.on_update))
                    changed = True
                out.append(inst)
            if changed:
                bb.instructions = out


def _build(mode, repeat):
    import concourse.bass as bass
    import concourse.mybir as mybir
    import concourse.tile as tile

    PatchedTileContext = tile.TileContext

    f32 = mybir.dt.float32
    bf16 = mybir.dt.bfloat16
    nc = bass.Bass()

    if mode == "f32":
        wl_d = nc.dram_tensor("wl", [128, NP, NCHUNK, 128], f32, kind="ExternalInput")
        xb_d = nc.dram_tensor("xb", [128, NB, B], f32, kind="ExternalInput")
    elif mode == "bf16":
        wh_d = nc.dram_tensor("wh", [128, NP, NCHUNK, 128], bf16, kind="ExternalInput")
        xh_d = nc.dram_tensor("xh", [128, NB, B], bf16, kind="ExternalInput")
    else:  # bf16x3
        wh_d = nc.dram_tensor("wh", [128, NP, NCHUNK, 128], bf16, kind="ExternalInput")
        wl_d = nc.dram_tensor("wl", [128, NP, NCHUNK, 128], bf16, kind="ExternalInput")
        xh_d = nc.dram_tensor("xh", [128, NB, B], bf16, kind="ExternalInput")
        xl_d = nc.dram_tensor("xl", [128, NB, B], bf16, kind="ExternalInput")
    out_d = nc.dram_tensor("out", [128, NP, B], f32, kind="ExternalOutput")

    with PatchedTileContext(nc) as tc:
        with (
            tc.tile_pool(name="xpool", bufs=1) as xpool,
            tc.tile_pool(name="wpool", bufs=3) as wpool,
            tc.tile_pool(name="opool", bufs=1) as opool,
            tc.tile_pool(name="psum", bufs=8, space="PSUM") as pspool,
        ):
            # x blocks resident for the whole kernel
            if mode == "f32":
                xb_s = xpool.tile([128, NB, B], f32, tag="xb")
                nc.sync.dma_start(xb_s[:], xb_d[:])
            else:
                xh_s = xpool.tile([128, NB, B], bf16, tag="xh")
                nc.sync.dma_start(xh_s[:], xh_d[:])
                if mode == "bf16x3":
                    xl_s = xpool.tile([128, NB, B], bf16, tag="xl")
                    nc.sync.dma_start(xl_s[:], xl_d[:])

            out_s = opool.tile([128, NP, B], f32)

            def body():
                for g in range(NP // WG):
                    if mode == "f32":
                        wt = wpool.tile([128, WG, NCHUNK, 128], f32, tag="w")
                        nc.sync.dma_start(wt[:], wl_d[:, g * WG:(g + 1) * WG])
                    else:
                        wt = wpool.tile([128, WG, NCHUNK, 128], bf16, tag="w")
                        nc.sync.dma_start(wt[:], wh_d[:, g * WG:(g + 1) * WG])
                        if mode == "bf16x3":
                            wt2 = wpool.tile([128, WG, NCHUNK, 128], bf16, tag="w2")
                            nc.sync.dma_start(wt2[:], wl_d[:, g * WG:(g + 1) * WG])

                    for pp in range(WG):
                        p = g * WG + pp
                        ps = pspool.tile([128, B], f32, tag="ps")
                        for j in range(NCHUNK):
                            if mode == "f32":
                                nc.tensor.matmul(
                                    ps[:], wt[:, pp, j, :], xb_s[:, p + j, :],
                                    start=(j == 0), stop=(j == NCHUNK - 1))
                            elif mode == "bf16":
                                nc.tensor.matmul(
                                    ps[:], wt[:, pp, j, :], xh_s[:, p + j, :],
                                    start=(j == 0), stop=(j == NCHUNK - 1))
                            else:
                                nc.tensor.matmul(
                                    ps[:], wt[:, pp, j, :], xh_s[:, p + j, :],
                                    start=(j == 0), stop=False)
                                nc.tensor.matmul(
                                    ps[:], wt[:, pp, j, :], xl_s[:, p + j, :],
                                    start=False, stop=False)
                                nc.tensor.matmul(
                                    ps[:], wt2[:, pp, j, :], xh_s[:, p + j, :],
                                    start=False, stop=(j == NCHUNK - 1))
                        nc.any.tensor_copy(out_s[:, p, :], ps[:])
                nc.sync.dma_start(out_d[:], out_s[:])

            if repeat == 1:
                body()
            else:
                with tc.For_i(0, repeat, 1):
                    body()

    _legalize_single_wait(nc)
    return nc


# ---------------------------------------------------------------- pjrt runner

def _make_runner(nc):
    import jax
    import concourse.mybir as mybir
    from concourse.bass2jax import (_bass_exec_p, install_neuronx_cc_hook,
                                    partition_id_tensor)
    from jax.experimental.shard_map import shard_map
    from jax.sharding import Mesh, PartitionSpec

    install_neuronx_cc_hook()
    partition_name = (nc.partition_id_tensor.name
                      if nc.partition_id_tensor else None)

    in_names, out_names, out_avals, zero_shapes = [], [], [], []
    for alloc in nc.m.functions[0].allocations:
        if not isinstance(alloc, mybir.MemoryLocationSet):
            continue
        name = alloc.memorylocations[0].name
        if alloc.kind == "ExternalInput":
            if name != partition_name:
                in_names.append(name)
        elif alloc.kind == "ExternalOutput":
            shape = tuple(alloc.tensor_shape)
            dtype = mybir.dt.np(alloc.dtype)
            out_names.append(name)
            out_avals.append(jax.core.ShapedArray(shape, dtype))
            zero_shapes.append((shape, dtype))
    n_params = len(in_names)
    all_names = in_names + out_names
    if partition_name is not None:
        all_names = all_names + [partition_name]
    donate = tuple(range(n_params, n_params + len(out_names)))

    def _body(*args):
        operands = list(args)
        if partition_name is not None:
            operands.append(partition_id_tensor())
        outs = _bass_exec_p.bind(
            *operands,
            out_avals=tuple(out_avals),
            in_names=tuple(all_names),
            out_names=tuple(out_names),
            lowering_input_output_aliases=(),
            sim_require_finite=True,
            sim_require_nnan=True,
            nc=nc,
        )
        return tuple(outs)

    devices = jax.devices()[:NCORES]
    mesh = Mesh(np.asarray(devices), ("core",))
    n_io = n_params + len(out_names)
    sharded = jax.jit(
        shard_map(_body, mesh=mesh,
                  in_specs=(PartitionSpec("core"),) * n_io,
                  out_specs=(PartitionSpec("core"),) * len(out_names),
                  check_rep=False),
        donate_argnums=donate, keep_unused=True)

    def run(in_maps):
        concat_in = [
            np.concatenate([np.asarray(in_maps[c][n]) for c in range(NCORES)],
                           axis=0)
            for n in in_names]
        concat_zeros = [np.zeros((NCORES * s[0], *s[1:]), d)
                        for (s, d) in zero_shapes]
        out_arrs = sharded(*concat_in, *concat_zeros)
        return [
            {n: np.asarray(out_arrs[i]).reshape(NCORES, *out_avals[i].shape)[c]
             for i, n in enumerate(out_names)}
            for c in range(NCORES)]

    run.jitted = sharded
    run.in_names = in_names
    run.zero_shapes = zero_shapes
    return run


def _get_runner(mode=None, repeat=None):
    mode = mode or MODE
    repeat = repeat or REPEAT
    key = (mode, repeat)
    if key not in _cache:
        nc = _build(mode, repeat)
        _cache[key] = _make_runner(nc)
    return _cache[key]


def kernel(x, weight):
    x = np.asarray(x, dtype=np.float32)
    weight = np.asarray(weight, dtype=np.float32)
    run = _get_runner()
    in_maps = _host_prep(x, weight, MODE)
    results = run(in_maps)
    return _host_post(results)
